# revision 18
# baseline (speedup 1.0000x reference)
"""DepthTransformer Trainium2 kernel, v2: channel-major attention with
PE-side reductions.

Data-parallel over b=6 across 8 cores (cores 6,7 duplicate; host drops).

Per core (one sample), all channel-major:
  ctxp = relu(gn_ctx(W_ctx @ ctx))            [64 j, 32 d, 1024 px] (pre-pos!)
  xp   = silu(gn_in(W_in @ x + b))            [320, 1024]
  qt   = Wqk @ xp, Wqk_n = s*Wk_n^T Wq_n      [(n,j) 512, 1024]
  sim[n,d,p] = sum_j qt[(n,j),p]*ctxp[j,d,p] + sum_j qt[(n,j),p]*pos[d,j]
    - pos term: PE matmul (stationary pos2), accumulated in PSUM
    - products: DVE TT bf16 (2x mode), prod[(s,j),(d,px)]
    - sum_j: PE ones-matmul (w2) into PSUM rows (d%16)*8+2t+s
  softmax over d: via small DMA transposes to pixel-major, DVE/ACT, back
  ctxa[n,j,p] = sum_d ctxp[j,d,p]*a[n,d,p] + sum_d pos[d,j]*a[n,d,p]
    - same trick: products on DVE with ctxd (d-major, 4x replicated via
      DRAM stride-0 gather), sums + pos term on PE into PSUM rows
      (j%16)*8+4q+m
  out1 = Wov @ ctxa (Wov column-permuted to ctxa's PSUM row layout)
  y = conv3x3(relu(gn2(conv3x3(relu(gn1(out1)), w1))), w2) + x
"""

import os
import numpy as np
import ml_dtypes

import concourse.bass as bass
import concourse.bacc as bacc
import concourse.tile as tile
from concourse import mybir
from concourse.bass_utils import run_bass_kernel_spmd

F32 = mybir.dt.float32
F16 = mybir.dt.float16
BF16 = mybir.dt.bfloat16
AF = mybir.ActivationFunctionType
ALU = mybir.AluOpType
AX = mybir.AxisListType

HN, HD, CD, D = 8, 40, 64, 32
CH = HN * HD          # 320
NPIX = 1024           # 32*32
NT = 3                # channel tiles of 128 for 320 (padded to 384)
EPS = 1e-5
DEBUG = bool(int(os.environ.get("DT_DEBUG", "0")))


def _bcast(ap, axis, n):
    """Insert a step-0 broadcast dim of size n at free-dim position `axis`."""
    return ap.unsqueeze(axis).broadcast_to(
        tuple(ap.shape[:axis]) + (n,) + tuple(ap.shape[axis:]))


def build_program():
    nc = bacc.Bacc("TRN2", target_bir_lowering=False, debug=False)

    def inp(name, shape, dt=F32):
        return nc.dram_tensor(name, shape, dt, kind="ExternalInput").ap()

    x_d = inp("x", [384, NPIX])
    xbf_d = inp("x_bf", [384, NPIX], BF16)
    ctx_d = inp("ctxin", [CD, D * NPIX], BF16)
    w_in_t = inp("w_in_t", [NT, NT, 128, 128], BF16)
    b_in = inp("b_in", [384, 1])
    gin_g = inp("gin_g", [384, 1])
    gin_b = inp("gin_b", [384, 1])
    wctx_t = inp("wctx_t", [CD, CD], BF16)
    gctx_g = inp("gctx_g", [CD, 1])
    gctx_b = inp("gctx_b", [CD, 1])
    wqkt_d = inp("wqkt", [NT, 128, 512], BF16)
    pos2_d = inp("pos2", [2, 4, 128, 128], BF16)
    pos4_d = inp("pos4", [2, 4, 128, 128], BF16)
    w2s_d = inp("w2s", [4, 4, 128, 32], BF16)
    w4s_d = inp("w4s", [2, 4, 128, 32], BF16)
    wovt_d = inp("wovt", [4, NT, 128, 128], BF16)
    g1_g = inp("g1_g", [384, 1])
    g1_b = inp("g1_b", [384, 1])
    g2_g = inp("g2_g", [384, 1])
    g2_b = inp("g2_b", [384, 1])
    c1_d = inp("conv1_t", [NT, NT, 128, 9, 128], BF16)
    c2_d = inp("conv2_t", [NT, NT, 128, 9, 128], BF16)
    gsel_d = inp("gsel", [NT, NT, 128, 128])
    g2c_d = inp("g2ctx", [CD, CD])

    y_d = nc.dram_tensor("y", [CH, NPIX], F32, kind="ExternalOutput").ap()

    # DRAM scratch
    dmaj_d = nc.dram_tensor("dmaj", [D * CD, NPIX], BF16).ap()   # row d*64+j
    sim_d = nc.dram_tensor("simd", [256, NPIX], F16).ap()        # (dh,d16,h)
    a_d = nc.dram_tensor("ad", [NPIX, 256], BF16).ap()           # px,(h,d)

    dbg = {}
    if DEBUG:
        for nm, shape, dt in [
            ("xp_dbg", [CH, NPIX], F32),
            ("ctxp_dbg", [CD, D * NPIX], BF16),
            ("qt_dbg", [512, NPIX], BF16),
            ("sim_dbg", [256, NPIX], F16),
            ("a_dbg", [NPIX, 256], BF16),
            ("ctxa_dbg", [512, NPIX], BF16),
            ("out1_dbg", [CH, NPIX], F32),
        ]:
            dbg[nm] = nc.dram_tensor(nm, shape, dt, kind="ExternalOutput").ap()

    with tile.TileContext(nc) as tc:
        from contextlib import ExitStack

        es = ExitStack()
        persist = es.enter_context(tc.tile_pool(name="persist", bufs=1))
        wpool = es.enter_context(tc.tile_pool(name="wpool", bufs=1))
        stage = es.enter_context(tc.tile_pool(name="stage", bufs=3))
        small = es.enter_context(tc.tile_pool(name="small", bufs=2))
        prodp = es.enter_context(tc.tile_pool(name="prodp", bufs=2))
        smp = es.enter_context(tc.tile_pool(name="smp", bufs=2))
        psum = es.enter_context(tc.tile_pool(name="psum", bufs=3, space="PSUM"))
        apsum = es.enter_context(tc.tile_pool(name="apsum", bufs=1, space="PSUM"))
        spsum = es.enter_context(tc.tile_pool(name="spsum", bufs=1, space="PSUM"))

        # ---------------- persistent weights ----------------
        _wn = [0]
        def load_w(pool, src, shape, dt=F32, tag=None):
            _wn[0] += 1
            t = pool.tile(shape, dt, tag=tag or "", name=f"w{_wn[0]}")
            nc.sync.dma_start(out=t[:], in_=src)
            return t

        win_sb = [[load_w(wpool, w_in_t[k, m], [128, 128], BF16) for m in range(NT)]
                  for k in range(NT)]
        wqkt_sb = [load_w(wpool, wqkt_d[k], [128, 512], BF16) for k in range(NT)]
        wctx_sb = load_w(wpool, wctx_t, [CD, CD], BF16)
        pos2_sb = [[load_w(wpool, pos2_d[dh, t], [128, 128], BF16) for t in range(4)]
                   for dh in range(2)]
        pos4_sb = [[load_w(wpool, pos4_d[q, jq], [128, 128], BF16) for jq in range(4)]
                   for q in range(2)]
        w2s_sb = [[load_w(wpool, w2s_d[t, d4], [128, 32], BF16) for d4 in range(4)]
                  for t in range(4)]
        w4s_sb = [[load_w(wpool, w4s_d[q, j4], [128, 32], BF16) for j4 in range(4)]
                  for q in range(2)]
        wovt_sb = [[load_w(wpool, wovt_d[k, m], [128, 128], BF16) for m in range(NT)]
                   for k in range(4)]
        gsel_sb = [[load_w(wpool, gsel_d[k, m], [128, 128]) for m in range(NT)]
                   for k in range(NT)]
        g2c_sb = load_w(wpool, g2c_d, [CD, CD])
        bin_sb = [load_w(wpool, b_in[m * 128:(m + 1) * 128], [128, 1]) for m in range(NT)]

        def load_vec(src):
            return [load_w(wpool, src[m * 128:(m + 1) * 128], [128, 1]) for m in range(NT)]

        gin_g_sb, gin_b_sb = load_vec(gin_g), load_vec(gin_b)
        g1_g_sb, g1_b_sb = load_vec(g1_g), load_vec(g1_b)
        g2_g_sb, g2_b_sb = load_vec(g2_g), load_vec(g2_b)
        gctx_g_sb = load_w(wpool, gctx_g, [CD, 1])
        gctx_b_sb = load_w(wpool, gctx_b, [CD, 1])

        eps_sb = wpool.tile([128, 1], F32)
        nc.vector.memset(eps_sb[:], EPS)

        # =========== GN stats helper (320-channel, 8 groups of 40) ===========
        def gn_affine_320(raw_tiles, gamma_sb, beta_sb, name):
            """raw_tiles: 3x [128,1024] f32 SBUF. Returns (s, t) per-tile [128,1]."""
            mv2 = []
            for t in range(NT):
                st = small.tile([128, 2, 6], F32, tag=f"{name}_st{t}", name=f"{name}_st{t}")
                for i in range(2):
                    nc.vector.bn_stats(out=st[:, i, :], in_=raw_tiles[t][:, i * 512:(i + 1) * 512])
                mv = small.tile([128, 2], F32, tag=f"{name}_mv{t}", name=f"{name}_mv{t}")
                nc.vector.bn_aggr(out=mv[:], in_=st[:])
                m2 = small.tile([128, 2], F32, tag=f"{name}_mv2{t}", name=f"{name}_m2{t}")
                nc.vector.tensor_copy(m2[:, 0:1], mv[:, 0:1])
                nc.vector.tensor_mul(m2[:, 1:2], mv[:, 0:1], mv[:, 0:1])
                nc.vector.tensor_add(m2[:, 1:2], m2[:, 1:2], mv[:, 1:2])
                mv2.append(m2)
            s_t = []
            for m in range(NT):
                gp = spsum.tile([128, 2], F32, tag="gnps")
                for k in range(NT):
                    nc.tensor.matmul(gp[:], gsel_sb[k][m][:], mv2[k][:],
                                     start=(k == 0), stop=(k == NT - 1))
                gs = small.tile([128, 2], F32, tag=f"{name}_gs{m}")
                nc.vector.tensor_copy(gs[:], gp[:])
                s = small.tile([128, 1], F32, tag=f"{name}_s{m}")
                tt = small.tile([128, 1], F32, tag=f"{name}_t{m}")
                vt = small.tile([128, 1], F32, tag=f"{name}_v")
                nc.vector.tensor_mul(vt[:], gs[:, 0:1], gs[:, 0:1])
                nc.vector.tensor_sub(vt[:], gs[:, 1:2], vt[:])
                nc.scalar.activation(out=vt[:], in_=vt[:], func=AF.Sqrt, bias=eps_sb[:, 0:1])
                nc.vector.reciprocal(out=vt[:], in_=vt[:])
                nc.vector.tensor_mul(s[:], gamma_sb[m][:], vt[:])
                nc.vector.tensor_mul(tt[:], gs[:, 0:1], s[:])
                nc.vector.tensor_sub(tt[:], beta_sb[m][:], tt[:])
                s_t.append((s, tt))
            return s_t

        # ---------------- phase A: ctx proj + gn stats ----------------
        # ctxp_rep rows 0-63: raw (then in-place relu'd) ctxp, free (d, px)
        ctxp_rep = persist.tile([128, D, NPIX], BF16, name="ctxp_rep")
        cst = persist.tile([CD, 64, 6], F32, tag="pad1_0", name="cst")
        for c in range(64):
            d, hf = c // 2, c % 2
            cin = stage.tile([CD, 512], BF16, tag="ctxin", bufs=4)
            nc.sync.dma_start(out=cin[:], in_=ctx_d[:, c * 512:(c + 1) * 512])
            ps = psum.tile([CD, 512], F32, tag="mm")
            nc.tensor.matmul(ps[:], wctx_sb[:], cin[:], start=True, stop=True)
            dst = ctxp_rep[0:64, d, hf * 512:(hf + 1) * 512]
            nc.scalar.activation(out=dst, in_=ps[:], func=AF.Copy)
            nc.vector.bn_stats(out=cst[:, c, :], in_=dst)
        cmv = small.tile([CD, 2], F32)
        nc.vector.bn_aggr(out=cmv[:], in_=cst[:])
        cmv2 = small.tile([CD, 2], F32)
        nc.vector.tensor_copy(cmv2[:, 0:1], cmv[:, 0:1])
        nc.vector.tensor_mul(cmv2[:, 1:2], cmv[:, 0:1], cmv[:, 0:1])
        nc.vector.tensor_add(cmv2[:, 1:2], cmv2[:, 1:2], cmv[:, 1:2])
        cgp = spsum.tile([CD, 2], F32, tag="gnps")
        nc.tensor.matmul(cgp[:], g2c_sb[:], cmv2[:], start=True, stop=True)
        cgs = small.tile([CD, 2], F32)
        nc.vector.tensor_copy(cgs[:], cgp[:])
        cs = small.tile([CD, 1], F32)
        ct = small.tile([CD, 1], F32)
        cv = small.tile([CD, 1], F32)
        nc.vector.tensor_mul(cv[:], cgs[:, 0:1], cgs[:, 0:1])
        nc.vector.tensor_sub(cv[:], cgs[:, 1:2], cv[:])
        nc.scalar.activation(out=cv[:], in_=cv[:], func=AF.Sqrt, bias=eps_sb[:CD, 0:1])
        nc.vector.reciprocal(out=cv[:], in_=cv[:])
        nc.vector.tensor_mul(cs[:], gctx_g_sb[:], cv[:])
        nc.vector.tensor_mul(ct[:], cgs[:, 0:1], cs[:])
        nc.vector.tensor_sub(ct[:], gctx_b_sb[:], ct[:])

        # ---------------- phase B: proj_in + gn_in + silu ----------------
        xbf_sb = [persist.tile([128, NPIX], BF16, name=f"xb_{t}") for t in range(NT)]
        for t in range(NT):
            nc.sync.dma_start(out=xbf_sb[t][:], in_=xbf_d[t * 128:(t + 1) * 128, :])

        xp = [persist.tile([128, NPIX], F32, tag=f"out1_{t}", name=f"xp_{t}") for t in range(NT)]
        xp_bf = [persist.tile([128, NPIX], BF16, tag=f"pad1_{t}", name=f"xpb_{t}") for t in range(NT)]
        for m in range(NT):
            for n in range(2):
                ps = psum.tile([128, 512], F32, tag="mm")
                for k in range(NT):
                    nc.tensor.matmul(ps[:], win_sb[k][m][:], xbf_sb[k][:, n * 512:(n + 1) * 512],
                                     start=(k == 0), stop=(k == NT - 1))
                nc.scalar.activation(out=xp[m][:, n * 512:(n + 1) * 512], in_=ps[:],
                                     func=AF.Identity, bias=bin_sb[m][:, 0:1])
        st_in = gn_affine_320(xp, gin_g_sb, gin_b_sb, "gin")
        for m in range(NT):
            s, t = st_in[m]
            nc.scalar.activation(out=xp_bf[m][:], in_=xp[m][:], func=AF.Silu,
                                 bias=t[:, 0:1], scale=s[:, 0:1])
        if DEBUG:
            for m in range(NT):
                hi = min(128, CH - m * 128)
                nc.sync.dma_start(out=dbg["xp_dbg"][m * 128:m * 128 + hi, :], in_=xp[m][:hi, :])

        # ---------------- phase C: qt channel-major ----------------
        qt = [persist.tile([128, NPIX], BF16, name=f"qt_{t}") for t in range(4)]
        for t in range(4):
            for hf in range(2):
                ps = psum.tile([128, 512], F32, tag="mm")
                for k in range(NT):
                    nc.tensor.matmul(ps[:], wqkt_sb[k][:, t * 128:(t + 1) * 128],
                                     xp_bf[k][:, hf * 512:(hf + 1) * 512],
                                     start=(k == 0), stop=(k == NT - 1))
                nc.scalar.activation(out=qt[t][:, hf * 512:(hf + 1) * 512], in_=ps[:],
                                     func=AF.Copy)
            if DEBUG:
                nc.sync.dma_start(out=dbg["qt_dbg"][t * 128:(t + 1) * 128, :], in_=qt[t][:])

        # ---------------- ctx apply: relu(s*x+t) in place, replicate ----------------
        for do in range(4):
            sl = ctxp_rep[0:64, do * 8:(do + 1) * 8, :]
            nc.scalar.activation(out=sl, in_=sl, func=AF.Relu,
                                 bias=ct[:, 0:1], scale=cs[:, 0:1])
        # write d-major DRAM layout (row d*64+j) for the ctxd gather
        odm = bass.AP(tensor=dmaj_d.tensor, offset=0,
                      ap=[[NPIX, CD], [CD * NPIX, D], [1, NPIX]])
        nc.sync.dma_start(out=odm, in_=ctxp_rep[0:64, :, :])
        # replicate rows 0-63 -> 64-127 (second copy for (s,j) product tiles)
        nc.gpsimd.dma_start(out=ctxp_rep[64:128, :, :], in_=ctxp_rep[0:64, :, :])
        if DEBUG:
            nc.sync.dma_start(out=dbg["ctxp_dbg"][:, :],
                              in_=ctxp_rep[0:64, :, :].rearrange("p d x -> p (d x)"))

        # ---------------- phase D: attention ----------------
        a_ch = [persist.tile([128, NPIX], BF16, name=f"ach_{q}") for q in range(2)]
        ctxa_ch = [persist.tile([128, NPIX], BF16, name=f"cxc_{jq}") for jq in range(4)]

        # --- sim (both px-halves): pos term + products + j-sums -> sim_d ---
        for h in range(2):
            pxs = slice(h * 512, (h + 1) * 512)
            ps_dh = [apsum.tile([128, 512], F32, tag=f"ap{i}", name=f"psdh{i}") for i in range(2)]
            for dh in range(2):
                for t in range(4):
                    nc.tensor.matmul(ps_dh[dh][:], pos2_sb[dh][t][:], qt[t][:, pxs],
                                     start=(t == 0), stop=False, skip_group_check=True)
            ctxp_s = ctxp_rep[:].rearrange("p (k f) x -> p f k x", f=4)
            for t in range(4):
                for d4 in range(4):
                    prod = prodp.tile([128, 8, 512], BF16, tag="prod", name="prodS")
                    nc.vector.tensor_tensor(
                        out=prod[:], in0=_bcast(qt[t][:, pxs], 1, 8),
                        in1=ctxp_s[:, d4, :, pxs], op=ALU.mult)
                    for k in range(8):  # d = d4 + 4k
                        b = k % 4
                        nc.tensor.matmul(
                            ps_dh[k // 4][b * 32:(b + 1) * 32, :],
                            w2s_sb[t][d4][:], prod[:, k, :],
                            start=False, stop=(t == 3 and d4 == 3 and b == 3),
                            skip_group_check=True, tile_position=(0, b * 32))
            for dh in range(2):
                simch = smp.tile([128, 512], F16, tag=f"simch{dh}", bufs=1)
                nc.scalar.activation(out=simch[:], in_=ps_dh[dh][:], func=AF.Copy)
                nc.sync.dma_start(out=sim_d[dh * 128:(dh + 1) * 128, pxs], in_=simch[:])
                if DEBUG:
                    nc.sync.dma_start(out=dbg["sim_dbg"][dh * 128:(dh + 1) * 128, pxs],
                                      in_=simch[:])

        # --- softmax pixel-major; a -> a_d -> a_ch ---
        for P in range(8):
            # cols after transpose: (dh, dmid, d4, h); d = 16*dh+4*dmid+d4
            sim_t = smp.tile([128, 2, 4, 4, 8], F16, tag="simt")
            flat = sim_t[:].rearrange("p a b c d -> p (a b c d)")
            for dh in range(2):
                nc.sync.dma_start_transpose(
                    out=flat[:, dh * 128:(dh + 1) * 128],
                    in_=sim_d[dh * 128:(dh + 1) * 128, P * 128:(P + 1) * 128])
            mx = smp.tile([128, 8], F32, tag="mx")
            nc.vector.tensor_reduce(
                out=mx[:], in_=sim_t[:].rearrange("p a b c h -> p h (a b c)"),
                axis=AX.X, op=ALU.max, negate=True)
            etmp = smp.tile([128, 8, 32], F32, tag="etmp", bufs=1)
            for n in range(8):
                nc.scalar.activation(
                    out=etmp[:, n, :],
                    in_=sim_t[:, :, :, :, n].rearrange("p a b c -> p (a b c)"),
                    func=AF.Exp, bias=mx[:, n:n + 1])
            sms = smp.tile([128, 8], F32, tag="sms")
            nc.vector.tensor_reduce(out=sms[:], in_=etmp[:], axis=AX.X, op=ALU.add)
            nc.vector.reciprocal(out=sms[:], in_=sms[:])
            a_pm = smp.tile([128, 8, 32], BF16, tag="apm", bufs=1)
            nc.vector.tensor_tensor(out=a_pm[:], in0=etmp[:],
                                    in1=_bcast(sms[:], 2, 32), op=ALU.mult)
            apf = a_pm[:].rearrange("p a b -> p (a b)")
            nc.sync.dma_start(out=a_d[P * 128:(P + 1) * 128, :], in_=apf)
            if DEBUG:
                nc.sync.dma_start(out=dbg["a_dbg"][P * 128:(P + 1) * 128, :], in_=apf)
        for q in range(2):
            for P in range(8):
                nc.sync.dma_start_transpose(
                    out=a_ch[q][:, P * 128:(P + 1) * 128],
                    in_=a_d[P * 128:(P + 1) * 128, q * 128:(q + 1) * 128])

        # --- ctxa per px-half (ctxd reuses the ctxp_rep slot) ---
        for h in range(2):
            pxs = slice(h * 512, (h + 1) * 512)
            ctxd = persist.tile([128, CD, 512], BF16, tag="ctxp_rep", name=f"ctxd{h}")
            idm = bass.AP(tensor=dmaj_d.tensor, offset=h * 512,
                          ap=[[0, 4], [CD * NPIX, D], [NPIX, CD], [1, 512]])
            nc.sync.dma_start(out=ctxd[:], in_=idm)
            ps_a = [apsum.tile([128, 512], F32, tag=f"ap{i}", name=f"psa{i}") for i in range(4)]
            for q in range(2):
                for jq in range(4):
                    nc.tensor.matmul(ps_a[jq][:], pos4_sb[q][jq][:], a_ch[q][:, pxs],
                                     start=(q == 0), stop=False, skip_group_check=True)
            ctxd_s = ctxd[:].rearrange("p (k f) x -> p f k x", f=4)
            for q in range(2):
                for j4 in range(4):
                    for c2 in range(2):
                        prod = prodp.tile([128, 8, 512], BF16, tag="prod", name="prodC")
                        nc.vector.tensor_tensor(
                            out=prod[:], in0=_bcast(a_ch[q][:, pxs], 1, 8),
                            in1=ctxd_s[:, j4, c2 * 8:(c2 + 1) * 8, :], op=ALU.mult)
                        for kk in range(8):  # j = j4 + 4k, k = c2*8+kk
                            k = c2 * 8 + kk
                            b = k % 4
                            nc.tensor.matmul(
                                ps_a[k // 4][b * 32:(b + 1) * 32, :],
                                w4s_sb[q][j4][:], prod[:, kk, :],
                                start=False, stop=(q == 1 and j4 == 3 and b == 3),
                                skip_group_check=True, tile_position=(0, b * 32))
            for jq in range(4):
                nc.scalar.activation(out=ctxa_ch[jq][:, pxs], in_=ps_a[jq][:],
                                     func=AF.Copy)
        if DEBUG:
            for jq in range(4):
                nc.sync.dma_start(out=dbg["ctxa_dbg"][jq * 128:(jq + 1) * 128, :],
                                  in_=ctxa_ch[jq][:])

        # ---------------- phase E: Wov -> out1 ----------------
        out1 = [persist.tile([128, NPIX], F32, tag=f"out1_{m}", name=f"out1_{m}") for m in range(NT)]
        for m in range(NT):
            for n in range(2):
                ps = psum.tile([128, 512], F32, tag="mm")
                for k in range(4):
                    nc.tensor.matmul(ps[:], wovt_sb[k][m][:],
                                     ctxa_ch[k][:, n * 512:(n + 1) * 512],
                                     start=(k == 0), stop=(k == 3))
                nc.scalar.activation(out=out1[m][:, n * 512:(n + 1) * 512], in_=ps[:], func=AF.Copy)
        if DEBUG:
            for m in range(NT):
                hi = min(128, CH - m * 128)
                nc.sync.dma_start(out=dbg["out1_dbg"][m * 128:m * 128 + hi, :], in_=out1[m][:hi, :])

        # ---------------- proj_out: gn1/relu/conv1, gn2/relu/conv2 ----------------
        st1 = gn_affine_320(out1, g1_g_sb, g1_b_sb, "gn1")
        pad1 = [persist.tile([128, 34, 34], BF16, tag=f"pad1_{m}", name=f"pad1_{m}") for m in range(NT)]
        for m in range(NT):
            nc.vector.memset(pad1[m][:], 0.0)
            s, t = st1[m]
            nc.scalar.activation(out=pad1[m][:, 1:33, 1:33],
                                 in_=out1[m][:].rearrange("p (h w) -> p h w", w=32),
                                 func=AF.Relu, bias=t[:, 0:1], scale=s[:, 0:1])

        def conv3x3(w_d, src_pad, name):
            cwt = persist.tile([128, 9, 9, 128], BF16, tag="ctxp_rep", name=f"{name}w")
            cw = [[cwt[:, k * 3 + m, :, :] for m in range(NT)] for k in range(NT)]
            for k in range(NT):
                for m in range(NT):
                    nc.sync.dma_start(out=cw[k][m], in_=w_d[k, m])
            out = [persist.tile([128, NPIX], F32, tag=f"out1_{m}", name=f"cv_{m}") for m in range(NT)]
            for m in range(NT):
                for n in range(2):
                    r0 = n * 16
                    ps = psum.tile([128, 512], F32, tag="mm")
                    first = True
                    for tap in range(9):
                        dy, dx = tap // 3, tap % 3
                        for k in range(NT):
                            nc.tensor.matmul(
                                ps[:], cw[k][m][:, tap, :],
                                src_pad[k][:, r0 + dy:r0 + dy + 16, dx:dx + 32],
                                start=first, stop=(tap == 8 and k == NT - 1))
                            first = False
                    nc.scalar.activation(out=out[m][:, n * 512:(n + 1) * 512], in_=ps[:],
                                         func=AF.Copy)
            return out

        y2 = conv3x3(c1_d, pad1, "c1")
        st2 = gn_affine_320(y2, g2_g_sb, g2_b_sb, "gn2")
        pad2 = [persist.tile([128, 34, 34], BF16, tag=f"pad1_{m}", name=f"pad2_{m}") for m in range(NT)]
        for m in range(NT):
            nc.vector.memset(pad2[m][:], 0.0)
            s, t = st2[m]
            nc.scalar.activation(out=pad2[m][:, 1:33, 1:33],
                                 in_=y2[m][:].rearrange("p (h w) -> p h w", w=32),
                                 func=AF.Relu, bias=t[:, 0:1], scale=s[:, 0:1])

        # conv2 + residual (x streamed back in)
        cwt2 = persist.tile([128, 9, 9, 128], BF16, tag="ctxp_rep", name="c2w")
        cw2 = [[cwt2[:, k * 3 + m, :, :] for m in range(NT)] for k in range(NT)]
        for k in range(NT):
            for m in range(NT):
                nc.sync.dma_start(out=cw2[k][m], in_=c2_d[k, m])
        for m in range(NT):
            hi = min(128, CH - m * 128)
            for n in range(2):
                r0 = n * 16
                ps = psum.tile([128, 512], F32, tag="mm")
                first = True
                for tap in range(9):
                    dy, dx = tap // 3, tap % 3
                    for k in range(NT):
                        nc.tensor.matmul(
                            ps[:], cw2[k][m][:, tap, :],
                            pad2[k][:, r0 + dy:r0 + dy + 16, dx:dx + 32],
                            start=first, stop=(tap == 8 and k == NT - 1))
                        first = False
                xres = stage.tile([128, 512], F32, tag="xres", bufs=2)
                nc.sync.dma_start(out=xres[:], in_=x_d[m * 128:(m + 1) * 128,
                                                       n * 512:(n + 1) * 512])
                fin = stage.tile([128, 512], F32, tag="fin", bufs=2)
                nc.vector.tensor_add(fin[:], ps[:], xres[:])
                nc.sync.dma_start(out=y_d[m * 128:m * 128 + hi, n * 512:(n + 1) * 512],
                                  in_=fin[:hi, :])
        es.close()

    nc.compile()
    return nc


_PROG = None
_LAST_RESULTS = None
_LAST_EXEC_NS = None


def _get_prog():
    global _PROG
    if _PROG is None:
        _PROG = build_program()
    return _PROG


def _prep_host(inputs):
    """Precompute folded weights; returns the common (weight) part of in_map."""
    f32 = np.float32
    bf16 = ml_dtypes.bfloat16
    w_in = np.asarray(inputs["w_in"], f32)
    wq = np.asarray(inputs["wq"], f32)
    wk = np.asarray(inputs["wk"], f32)
    wv = np.asarray(inputs["wv"], f32)
    wout = np.asarray(inputs["w_attn_out"], f32)
    pos = np.asarray(inputs["pos_emb"], f32)   # [32 d, 64 c]
    scale = HD ** -0.5

    def pad_to(a, shape):
        out = np.zeros(shape, a.dtype)
        out[tuple(slice(0, s) for s in a.shape)] = a
        return out

    def tile_km(mat_t, kt, mt):  # mat_t: [K, M] -> [kt, mt, 128, 128]
        p = pad_to(mat_t, (kt * 128, mt * 128))
        return np.ascontiguousarray(
            p.reshape(kt, 128, mt, 128).transpose(0, 2, 1, 3))

    w_in_tiles = tile_km(w_in.T, NT, NT).astype(bf16)

    wqk = np.concatenate(
        [scale * (wk[n * HD:(n + 1) * HD, :].T @ wq[n * HD:(n + 1) * HD, :])
         for n in range(HN)], axis=0)          # [512, 320]
    wqkt = pad_to(wqk.T, (NT * 128, 512)).reshape(NT, 128, 512).astype(bf16)

    # sim PSUM row (in tile dh) = 32*((d%16)//4) + 8*(d%4) + 2t + s
    pos2 = np.zeros((2, 4, 128, 128), f32)
    for dh in range(2):
        for t in range(4):
            for s in range(2):
                for d16 in range(16):
                    col = 32 * (d16 // 4) + 8 * (d16 % 4) + 2 * t + s
                    pos2[dh, t, s * 64:(s + 1) * 64, col] = pos[dh * 16 + d16, :]
    # ctxa PSUM row (in tile jq) = 32*((j%16)//4) + 8*(j%4) + 4q + m
    pos4 = np.zeros((2, 4, 128, 128), f32)
    for q in range(2):
        for jq in range(4):
            for m in range(4):
                for j16 in range(16):
                    col = 32 * (j16 // 4) + 8 * (j16 % 4) + 4 * q + m
                    pos4[q, jq, m * 32:(m + 1) * 32, col] = pos[:, jq * 16 + j16]
    # reduce stationaries: block-internal row placement baked into columns
    w2s = np.zeros((4, 4, 128, 32), f32)
    for t in range(4):
        for d4 in range(4):
            for s in range(2):
                w2s[t, d4, s * 64:(s + 1) * 64, d4 * 8 + 2 * t + s] = 1.0
    w4s = np.zeros((2, 4, 128, 32), f32)
    for q in range(2):
        for j4 in range(4):
            for r in range(4):
                w4s[q, j4, r * 32:(r + 1) * 32, j4 * 8 + 4 * q + r] = 1.0

    wov = np.concatenate(
        [wout[:, n * HD:(n + 1) * HD] @ wv[n * HD:(n + 1) * HD, :]
         for n in range(HN)], axis=1)          # [320, 512] cols (n, j)
    # permute cols to ctxa PSUM row layout:
    # k = jq*128 + jmid*32 + j4*8 + 4q + m <-> (n=4q+m, j=jq*16+jmid*4+j4)
    idx = np.zeros(512, np.int64)
    for jq in range(4):
        for jmid in range(4):
            for j4 in range(4):
                for q in range(2):
                    for m in range(4):
                        idx[jq * 128 + jmid * 32 + j4 * 8 + 4 * q + m] = \
                            (4 * q + m) * 64 + jq * 16 + jmid * 4 + j4
    wov_re = wov[:, idx]
    wov_tiles = tile_km(wov_re.T, 4, NT).astype(bf16)

    def conv_tiles(w):  # [o, i, 3, 3] -> [kt, mt, 128, 9, 128]
        taps = np.stack([tile_km(np.ascontiguousarray(w[:, :, t // 3, t % 3].T), NT, NT)
                         for t in range(9)], axis=0)
        return np.ascontiguousarray(taps.transpose(1, 2, 3, 0, 4)).astype(bf16)

    gsel = np.zeros((CH, CH), f32)
    for g in range(8):
        gsel[g * 40:(g + 1) * 40, g * 40:(g + 1) * 40] = 1.0 / 40
    g2ctx = np.zeros((CD, CD), f32)
    for g in range(8):
        g2ctx[g * 8:(g + 1) * 8, g * 8:(g + 1) * 8] = 1.0 / 8

    def col(v):
        return pad_to(np.asarray(v, f32).reshape(-1, 1), (384, 1))

    common = {
        "w_in_t": w_in_tiles,
        "b_in": col(inputs["b_in"]),
        "gin_g": col(inputs["gn_in_g"]), "gin_b": col(inputs["gn_in_b"]),
        "wctx_t": np.ascontiguousarray(np.asarray(inputs["w_ctx"], f32).T).astype(bf16),
        "gctx_g": np.asarray(inputs["gn_ctx_g"], f32).reshape(CD, 1),
        "gctx_b": np.asarray(inputs["gn_ctx_b"], f32).reshape(CD, 1),
        "wqkt": wqkt,
        "pos2": pos2.astype(bf16),
        "pos4": pos4.astype(bf16),
        "w2s": w2s.astype(bf16),
        "w4s": w4s.astype(bf16),
        "wovt": wov_tiles,
        "g1_g": col(inputs["gn1_g"]), "g1_b": col(inputs["gn1_b"]),
        "g2_g": col(inputs["gn2_g"]), "g2_b": col(inputs["gn2_b"]),
        "conv1_t": conv_tiles(np.asarray(inputs["conv1_w"], f32)),
        "conv2_t": conv_tiles(np.asarray(inputs["conv2_w"], f32)),
        "gsel": tile_km(gsel, NT, NT),
        "g2ctx": g2ctx,
    }
    return common


def kernel(**inputs):
    nc = _get_prog()
    common = _prep_host(inputs)
    x = np.asarray(inputs["x"], np.float32)      # [6, 320, 32, 32]
    ctx = np.asarray(inputs["context"], np.float32)  # [6, 64, 32, 32, 32]
    b = x.shape[0]
    in_maps = []
    for core in range(8):
        s = core if core < b else core - b
        m = dict(common)
        xs = np.zeros((384, NPIX), np.float32)
        xs[:CH] = x[s].reshape(CH, NPIX)
        m["x"] = xs
        m["x_bf"] = xs.astype(ml_dtypes.bfloat16)
        m["ctxin"] = np.ascontiguousarray(
            ctx[s].reshape(CD, D * NPIX)).astype(ml_dtypes.bfloat16)
        in_maps.append(m)
    trace = bool(int(os.environ.get("DT_TRACE", "0")))
    kw = {}
    if trace:
        import sys
        import types
        try:
            import antenv.axon_hooks  # noqa: F401
        except ImportError:
            from trn_agent_boot.trn_boot import _ntff_profile_via_ctypes
            mm = types.ModuleType("antenv.axon_hooks")
            _h = _ntff_profile_via_ctypes("/opt/axon/libaxon_pjrt.so")
            mm.get_axon_ntff_profile_hook = lambda: _h
            sys.modules["antenv.axon_hooks"] = mm
        kw = dict(trace=True, tmpdir=os.environ.get("DT_TRACE_DIR") or None)
    res = run_bass_kernel_spmd(nc, in_maps, list(range(8)), **kw)
    global _LAST_RESULTS, _LAST_EXEC_NS
    _LAST_RESULTS = res.results
    _LAST_EXEC_NS = res.exec_time_ns
    if trace:
        print(f"HW exec time: {res.exec_time_ns} ns")
    out = np.stack([res.results[s]["y"] for s in range(b)], axis=0)
    return out.reshape(b, CH, 32, 32).astype(np.float32)


if __name__ == "__main__":
    pass


# revision 22
# speedup vs baseline: 1.0415x; 1.0415x over previous
"""DepthTransformer Trainium2 kernel, v2: channel-major attention with
PE-side reductions.

Data-parallel over b=6 across 8 cores (cores 6,7 duplicate; host drops).

Per core (one sample), all channel-major:
  ctxp = relu(gn_ctx(W_ctx @ ctx))            [64 j, 32 d, 1024 px] (pre-pos!)
  xp   = silu(gn_in(W_in @ x + b))            [320, 1024]
  qt   = Wqk @ xp, Wqk_n = s*Wk_n^T Wq_n      [(n,j) 512, 1024]
  sim[n,d,p] = sum_j qt[(n,j),p]*ctxp[j,d,p] + sum_j qt[(n,j),p]*pos[d,j]
    - pos term: PE matmul (stationary pos2), accumulated in PSUM
    - products: DVE TT bf16 (2x mode), prod[(s,j),(d,px)]
    - sum_j: PE ones-matmul (w2) into PSUM rows (d%16)*8+2t+s
  softmax over d: via small DMA transposes to pixel-major, DVE/ACT, back
  ctxa[n,j,p] = sum_d ctxp[j,d,p]*a[n,d,p] + sum_d pos[d,j]*a[n,d,p]
    - same trick: products on DVE with ctxd (d-major, 4x replicated via
      DRAM stride-0 gather), sums + pos term on PE into PSUM rows
      (j%16)*8+4q+m
  out1 = Wov @ ctxa (Wov column-permuted to ctxa's PSUM row layout)
  y = conv3x3(relu(gn2(conv3x3(relu(gn1(out1)), w1))), w2) + x
"""

import os
import numpy as np
import ml_dtypes

import concourse.bass as bass
import concourse.bacc as bacc
import concourse.tile as tile
from concourse import mybir
from concourse.bass_utils import run_bass_kernel_spmd

F32 = mybir.dt.float32
F16 = mybir.dt.float16
BF16 = mybir.dt.bfloat16
AF = mybir.ActivationFunctionType
ALU = mybir.AluOpType
AX = mybir.AxisListType

HN, HD, CD, D = 8, 40, 64, 32
CH = HN * HD          # 320
NPIX = 1024           # 32*32
NT = 3                # channel tiles of 128 for 320 (padded to 384)
EPS = 1e-5
DEBUG = bool(int(os.environ.get("DT_DEBUG", "0")))


def _bcast(ap, axis, n):
    """Insert a step-0 broadcast dim of size n at free-dim position `axis`."""
    return ap.unsqueeze(axis).broadcast_to(
        tuple(ap.shape[:axis]) + (n,) + tuple(ap.shape[axis:]))


def build_program():
    nc = bacc.Bacc("TRN2", target_bir_lowering=False, debug=False)

    def inp(name, shape, dt=F32):
        return nc.dram_tensor(name, shape, dt, kind="ExternalInput").ap()

    x_d = inp("x", [384, NPIX])
    xbf_d = inp("x_bf", [384, NPIX], BF16)
    ctx_d = inp("ctxin", [CD, D * NPIX], BF16)
    w_in_t = inp("w_in_t", [NT, NT, 128, 128], BF16)
    b_in = inp("b_in", [384, 1])
    gin_g = inp("gin_g", [384, 1])
    gin_b = inp("gin_b", [384, 1])
    wctx_t = inp("wctx_t", [CD, CD], BF16)
    gctx_g = inp("gctx_g", [CD, 1])
    gctx_b = inp("gctx_b", [CD, 1])
    wqkt_d = inp("wqkt", [NT, 128, 512], BF16)
    pos2_d = inp("pos2", [2, 4, 128, 128], BF16)
    pos4_d = inp("pos4", [2, 4, 128, 128], BF16)
    w2s_d = inp("w2s", [4, 4, 128, 32], BF16)
    w4s_d = inp("w4s", [2, 4, 128, 32], BF16)
    wovt_d = inp("wovt", [4, NT, 128, 128], BF16)
    g1_g = inp("g1_g", [384, 1])
    g1_b = inp("g1_b", [384, 1])
    g2_g = inp("g2_g", [384, 1])
    g2_b = inp("g2_b", [384, 1])
    c1_d = inp("conv1_t", [NT, NT, 128, 9, 128], BF16)
    c2_d = inp("conv2_t", [NT, NT, 128, 9, 128], BF16)
    gsel_d = inp("gsel", [NT, NT, 128, 128])
    g2c_d = inp("g2ctx", [CD, CD])

    y_d = nc.dram_tensor("y", [CH, NPIX], F32, kind="ExternalOutput").ap()

    # DRAM scratch: ctxp in d-major layout, row (h*32+d), cols (j, px-half)
    ctxdd_d = nc.dram_tensor("ctxdd", [2 * D, CD * 512], BF16).ap()

    dbg = {}
    if DEBUG:
        for nm, shape, dt in [
            ("xp_dbg", [CH, NPIX], F32),
            ("ctxp_dbg", [CD, D * NPIX], BF16),
            ("qt_dbg", [512, NPIX], BF16),
            ("sim_dbg", [256, NPIX], F16),
            ("a_dbg", [NPIX, 256], BF16),
            ("ctxa_dbg", [512, NPIX], BF16),
            ("out1_dbg", [CH, NPIX], F32),
        ]:
            dbg[nm] = nc.dram_tensor(nm, shape, dt, kind="ExternalOutput").ap()

    with tile.TileContext(nc) as tc:
        from contextlib import ExitStack

        es = ExitStack()
        persist = es.enter_context(tc.tile_pool(name="persist", bufs=1))
        wpool = es.enter_context(tc.tile_pool(name="wpool", bufs=1))
        stage = es.enter_context(tc.tile_pool(name="stage", bufs=3))
        small = es.enter_context(tc.tile_pool(name="small", bufs=2))
        prodp = es.enter_context(tc.tile_pool(name="prodp", bufs=2))
        smp = es.enter_context(tc.tile_pool(name="smp", bufs=2))
        psum = es.enter_context(tc.tile_pool(name="psum", bufs=3, space="PSUM"))
        apsum = es.enter_context(tc.tile_pool(name="apsum", bufs=1, space="PSUM"))
        spsum = es.enter_context(tc.tile_pool(name="spsum", bufs=1, space="PSUM"))

        # ---------------- persistent weights ----------------
        _wn = [0]
        def load_w(pool, src, shape, dt=F32, tag=None):
            _wn[0] += 1
            t = pool.tile(shape, dt, tag=tag or "", name=f"w{_wn[0]}")
            nc.scalar.dma_start(out=t[:], in_=src)
            return t

        win_sb = [[load_w(wpool, w_in_t[k, m], [128, 128], BF16) for m in range(NT)]
                  for k in range(NT)]
        wqkt_sb = [load_w(wpool, wqkt_d[k], [128, 512], BF16) for k in range(NT)]
        wctx_sb = load_w(wpool, wctx_t, [CD, CD], BF16)
        pos2_sb = [[load_w(wpool, pos2_d[dh, t], [128, 128], BF16) for t in range(4)]
                   for dh in range(2)]
        pos4_sb = [[load_w(wpool, pos4_d[q, jq], [128, 128], BF16) for jq in range(4)]
                   for q in range(2)]
        w2s_sb = [[load_w(wpool, w2s_d[t, d4], [128, 32], BF16) for d4 in range(4)]
                  for t in range(4)]
        w4s_sb = [[load_w(wpool, w4s_d[q, j4], [128, 32], BF16) for j4 in range(4)]
                  for q in range(2)]
        wovt_sb = [[load_w(wpool, wovt_d[k, m], [128, 128], BF16) for m in range(NT)]
                   for k in range(4)]
        gsel_sb = [[load_w(wpool, gsel_d[k, m], [128, 128]) for m in range(NT)]
                   for k in range(NT)]
        g2c_sb = load_w(wpool, g2c_d, [CD, CD])
        bin_sb = [load_w(wpool, b_in[m * 128:(m + 1) * 128], [128, 1]) for m in range(NT)]

        def load_vec(src):
            return [load_w(wpool, src[m * 128:(m + 1) * 128], [128, 1]) for m in range(NT)]

        gin_g_sb, gin_b_sb = load_vec(gin_g), load_vec(gin_b)
        g1_g_sb, g1_b_sb = load_vec(g1_g), load_vec(g1_b)
        g2_g_sb, g2_b_sb = load_vec(g2_g), load_vec(g2_b)
        gctx_g_sb = load_w(wpool, gctx_g, [CD, 1])
        gctx_b_sb = load_w(wpool, gctx_b, [CD, 1])

        eps_sb = wpool.tile([128, 1], F32)
        nc.vector.memset(eps_sb[:], EPS)

        # =========== GN stats helper (320-channel, 8 groups of 40) ===========
        def gn_affine_320(raw_tiles, gamma_sb, beta_sb, name):
            """raw_tiles: 3x [128,1024] f32 SBUF. Returns (s, t) per-tile [128,1]."""
            mv2 = []
            for t in range(NT):
                st = small.tile([128, 2, 6], F32, tag=f"{name}_st{t}", name=f"{name}_st{t}")
                for i in range(2):
                    nc.vector.bn_stats(out=st[:, i, :], in_=raw_tiles[t][:, i * 512:(i + 1) * 512])
                mv = small.tile([128, 2], F32, tag=f"{name}_mv{t}", name=f"{name}_mv{t}")
                nc.vector.bn_aggr(out=mv[:], in_=st[:])
                m2 = small.tile([128, 2], F32, tag=f"{name}_mv2{t}", name=f"{name}_m2{t}")
                nc.vector.tensor_copy(m2[:, 0:1], mv[:, 0:1])
                nc.vector.tensor_mul(m2[:, 1:2], mv[:, 0:1], mv[:, 0:1])
                nc.vector.tensor_add(m2[:, 1:2], m2[:, 1:2], mv[:, 1:2])
                mv2.append(m2)
            s_t = []
            for m in range(NT):
                gp = spsum.tile([128, 2], F32, tag="gnps")
                for k in range(NT):
                    nc.tensor.matmul(gp[:], gsel_sb[k][m][:], mv2[k][:],
                                     start=(k == 0), stop=(k == NT - 1))
                gs = small.tile([128, 2], F32, tag=f"{name}_gs{m}")
                nc.vector.tensor_copy(gs[:], gp[:])
                s = small.tile([128, 1], F32, tag=f"{name}_s{m}")
                tt = small.tile([128, 1], F32, tag=f"{name}_t{m}")
                vt = small.tile([128, 1], F32, tag=f"{name}_v")
                nc.vector.tensor_mul(vt[:], gs[:, 0:1], gs[:, 0:1])
                nc.vector.tensor_sub(vt[:], gs[:, 1:2], vt[:])
                nc.scalar.activation(out=vt[:], in_=vt[:], func=AF.Sqrt, bias=eps_sb[:, 0:1])
                nc.vector.reciprocal(out=vt[:], in_=vt[:])
                nc.vector.tensor_mul(s[:], gamma_sb[m][:], vt[:])
                nc.vector.tensor_mul(tt[:], gs[:, 0:1], s[:])
                nc.vector.tensor_sub(tt[:], beta_sb[m][:], tt[:])
                s_t.append((s, tt))
            return s_t

        # ---------------- phase A: ctx proj + gn stats ----------------
        # ctxp_rep rows 0-63: raw (then in-place relu'd) ctxp, free (d, px)
        ctxp_rep = persist.tile([128, D, NPIX], BF16, name="ctxp_rep")
        cst = persist.tile([CD, 64, 6], F32, tag="pad1_0", name="cst")
        for c in range(64):
            d, hf = c // 2, c % 2
            cin = stage.tile([CD, 512], BF16, tag="ctxin", bufs=4)
            nc.sync.dma_start(out=cin[:], in_=ctx_d[:, c * 512:(c + 1) * 512])
            ps = psum.tile([CD, 512], F32, tag="mm")
            nc.tensor.matmul(ps[:], wctx_sb[:], cin[:], start=True, stop=True)
            dst = ctxp_rep[0:64, d, hf * 512:(hf + 1) * 512]
            nc.scalar.activation(out=dst, in_=ps[:], func=AF.Copy)
            nc.vector.bn_stats(out=cst[:, c, :], in_=dst)
        cmv = small.tile([CD, 2], F32)
        nc.vector.bn_aggr(out=cmv[:], in_=cst[:])
        cmv2 = small.tile([CD, 2], F32)
        nc.vector.tensor_copy(cmv2[:, 0:1], cmv[:, 0:1])
        nc.vector.tensor_mul(cmv2[:, 1:2], cmv[:, 0:1], cmv[:, 0:1])
        nc.vector.tensor_add(cmv2[:, 1:2], cmv2[:, 1:2], cmv[:, 1:2])
        cgp = spsum.tile([CD, 2], F32, tag="gnps")
        nc.tensor.matmul(cgp[:], g2c_sb[:], cmv2[:], start=True, stop=True)
        cgs = small.tile([CD, 2], F32)
        nc.vector.tensor_copy(cgs[:], cgp[:])
        cs = small.tile([CD, 1], F32)
        ct = small.tile([CD, 1], F32)
        cv = small.tile([CD, 1], F32)
        nc.vector.tensor_mul(cv[:], cgs[:, 0:1], cgs[:, 0:1])
        nc.vector.tensor_sub(cv[:], cgs[:, 1:2], cv[:])
        nc.scalar.activation(out=cv[:], in_=cv[:], func=AF.Sqrt, bias=eps_sb[:CD, 0:1])
        nc.vector.reciprocal(out=cv[:], in_=cv[:])
        nc.vector.tensor_mul(cs[:], gctx_g_sb[:], cv[:])
        nc.vector.tensor_mul(ct[:], cgs[:, 0:1], cs[:])
        nc.vector.tensor_sub(ct[:], gctx_b_sb[:], ct[:])

        # ---------------- phase B: proj_in + gn_in + silu ----------------
        xbf_sb = [persist.tile([128, NPIX], BF16, name=f"xb_{t}") for t in range(NT)]
        for t in range(NT):
            nc.sync.dma_start(out=xbf_sb[t][:], in_=xbf_d[t * 128:(t + 1) * 128, :])

        xp = [persist.tile([128, NPIX], F32, tag=f"out1_{t}", name=f"xp_{t}") for t in range(NT)]
        xp_bf = [persist.tile([128, NPIX], BF16, tag=f"pad1_{t}", name=f"xpb_{t}") for t in range(NT)]
        for m in range(NT):
            for n in range(2):
                ps = psum.tile([128, 512], F32, tag="mm")
                for k in range(NT):
                    nc.tensor.matmul(ps[:], win_sb[k][m][:], xbf_sb[k][:, n * 512:(n + 1) * 512],
                                     start=(k == 0), stop=(k == NT - 1))
                nc.scalar.activation(out=xp[m][:, n * 512:(n + 1) * 512], in_=ps[:],
                                     func=AF.Identity, bias=bin_sb[m][:, 0:1])
        st_in = gn_affine_320(xp, gin_g_sb, gin_b_sb, "gin")
        for m in range(NT):
            s, t = st_in[m]
            nc.scalar.activation(out=xp_bf[m][:], in_=xp[m][:], func=AF.Silu,
                                 bias=t[:, 0:1], scale=s[:, 0:1])
        if DEBUG:
            for m in range(NT):
                hi = min(128, CH - m * 128)
                nc.sync.dma_start(out=dbg["xp_dbg"][m * 128:m * 128 + hi, :], in_=xp[m][:hi, :])

        # ---------------- phase C: qt channel-major ----------------
        qt = [persist.tile([128, NPIX], BF16, name=f"qt_{t}") for t in range(4)]
        for t in range(4):
            for hf in range(2):
                ps = psum.tile([128, 512], F32, tag="mm")
                for k in range(NT):
                    nc.tensor.matmul(ps[:], wqkt_sb[k][:, t * 128:(t + 1) * 128],
                                     xp_bf[k][:, hf * 512:(hf + 1) * 512],
                                     start=(k == 0), stop=(k == NT - 1))
                nc.scalar.activation(out=qt[t][:, hf * 512:(hf + 1) * 512], in_=ps[:],
                                     func=AF.Copy)
            if DEBUG:
                nc.sync.dma_start(out=dbg["qt_dbg"][t * 128:(t + 1) * 128, :], in_=qt[t][:])

        # ---------------- ctx apply: relu(s*x+t) in place, replicate ----------------
        for do in range(4):
            sl = ctxp_rep[0:64, do * 8:(do + 1) * 8, :]
            nc.scalar.activation(out=sl, in_=sl, func=AF.Relu,
                                 bias=ct[:, 0:1], scale=cs[:, 0:1])
        # write d-major DRAM layout, row (h*32+d), cols (j, px-half)
        # ctxdd addr(g,d,j,px') = (g*32+d)*32768 + j*512 + px'
        ctxp_v = ctxp_rep[0:64, :, :].rearrange("p d (g x) -> p d g x", g=2)
        for jh in range(2):
            for g in range(2):
                odm = bass.AP(tensor=ctxdd_d.tensor,
                              offset=g * D * CD * 512 + jh * 32 * 512,
                              ap=[[512, 32], [CD * 512, D], [1, 512]])
                eng = nc.sync if jh == 0 else nc.gpsimd
                eng.dma_start(out=odm, in_=ctxp_v[jh * 32:(jh + 1) * 32, :, g, :])
        # replicate rows 0-63 -> 64-127 (second copy for (s,j) product tiles)
        nc.scalar.dma_start(out=ctxp_rep[64:128, :, :], in_=ctxp_rep[0:64, :, :])
        if DEBUG:
            nc.sync.dma_start(out=dbg["ctxp_dbg"][:, :],
                              in_=ctxp_rep[0:64, :, :].rearrange("p d x -> p (d x)"))

        # ---------------- phase D: attention ----------------
        a_ch = [persist.tile([128, NPIX], BF16, name=f"ach_{q}") for q in range(2)]
        ctxa_ch = [persist.tile([128, NPIX], BF16, name=f"cxc_{jq}") for jq in range(4)]

        # --- sim (both px-halves): pos term + products + j-sums ---
        simch = [[None, None], [None, None]]
        for h in range(2):
            pxs = slice(h * 512, (h + 1) * 512)
            ps_dh = [apsum.tile([128, 512], F32, tag=f"ap{i}", name=f"psdh{i}") for i in range(2)]
            for dh in range(2):
                for t in range(4):
                    nc.tensor.matmul(ps_dh[dh][:], pos2_sb[dh][t][:], qt[t][:, pxs],
                                     start=(t == 0), stop=False, skip_group_check=True)
            ctxp_s = ctxp_rep[:].rearrange("p (k f) x -> p f k x", f=4)
            for t in range(4):
                for d4 in range(4):
                    prod = prodp.tile([128, 8, 512], BF16, tag="prod", name="prodS")
                    nc.vector.tensor_tensor(
                        out=prod[:], in0=_bcast(qt[t][:, pxs], 1, 8),
                        in1=ctxp_s[:, d4, :, pxs], op=ALU.mult)
                    for k in range(8):  # d = d4 + 4k
                        b = k % 4
                        nc.tensor.matmul(
                            ps_dh[k // 4][b * 32:(b + 1) * 32, :],
                            w2s_sb[t][d4][:], prod[:, k, :],
                            start=False, stop=(t == 3 and d4 == 3 and b == 3),
                            skip_group_check=True, tile_position=(0, b * 32))
            for dh in range(2):
                sc = smp.tile([128, 512], F16, tag=f"simch{h}{dh}", bufs=1,
                              name=f"simch{h}{dh}")
                simch[h][dh] = sc
                nc.scalar.activation(out=sc[:], in_=ps_dh[dh][:], func=AF.Copy)
                if DEBUG:
                    nc.sync.dma_start(out=dbg["sim_dbg"][dh * 128:(dh + 1) * 128, pxs],
                                      in_=sc[:])

        # --- softmax pixel-major; a -> a_d -> a_ch ---
        for P in range(8):
            # cols after transpose: (dh, dmid, d4, h); d = 16*dh+4*dmid+d4
            sim_t = smp.tile([128, 2, 4, 4, 8], F16, tag="simt")
            flat = sim_t[:].rearrange("p a b c d -> p (a b c d)")
            for dh in range(2):
                nc.sync.dma_start_transpose(
                    out=flat[:, dh * 128:(dh + 1) * 128],
                    in_=simch[P // 4][dh][:, (P % 4) * 128:(P % 4 + 1) * 128])
            mx = smp.tile([128, 8], F32, tag="mx")
            nc.vector.tensor_reduce(
                out=mx[:], in_=sim_t[:].rearrange("p a b c h -> p h (a b c)"),
                axis=AX.X, op=ALU.max, negate=True)
            etmp = smp.tile([128, 8, 32], F32, tag="etmp", bufs=1)
            for n in range(8):
                nc.scalar.activation(
                    out=etmp[:, n, :],
                    in_=sim_t[:, :, :, :, n].rearrange("p a b c -> p (a b c)"),
                    func=AF.Exp, bias=mx[:, n:n + 1])
            sms = smp.tile([128, 8], F32, tag="sms")
            nc.vector.tensor_reduce(out=sms[:], in_=etmp[:], axis=AX.X, op=ALU.add)
            nc.vector.reciprocal(out=sms[:], in_=sms[:])
            a_pm = smp.tile([128, 8, 32], BF16, tag="apm", bufs=2)
            nc.vector.tensor_tensor(out=a_pm[:], in0=etmp[:],
                                    in1=_bcast(sms[:], 2, 32), op=ALU.mult)
            apf = a_pm[:].rearrange("p a b -> p (a b)")
            if DEBUG:
                nc.sync.dma_start(out=dbg["a_dbg"][P * 128:(P + 1) * 128, :], in_=apf)
            for q in range(2):
                nc.sync.dma_start_transpose(
                    out=a_ch[q][:, P * 128:(P + 1) * 128],
                    in_=apf[:, q * 128:(q + 1) * 128])

        # --- ctxa per px-half (ctxd reuses the ctxp_rep slot) ---
        for h in range(2):
            pxs = slice(h * 512, (h + 1) * 512)
            ctxd = persist.tile([128, CD, 512], BF16, tag="ctxp_rep", name=f"ctxd{h}")
            idm = bass.AP(tensor=ctxdd_d.tensor, offset=h * D * CD * 512,
                          ap=[[0, 4], [CD * 512, D], [1, CD * 512]])
            nc.scalar.dma_start(out=ctxd[:], in_=idm)
            ps_a = [apsum.tile([128, 512], F32, tag=f"ap{i}", name=f"psa{i}") for i in range(4)]
            for q in range(2):
                for jq in range(4):
                    nc.tensor.matmul(ps_a[jq][:], pos4_sb[q][jq][:], a_ch[q][:, pxs],
                                     start=(q == 0), stop=False, skip_group_check=True)
            ctxd_s = ctxd[:].rearrange("p (k f) x -> p f k x", f=4)
            for q in range(2):
                for j4 in range(4):
                    for c2 in range(2):
                        prod = prodp.tile([128, 8, 512], BF16, tag="prod", name="prodC")
                        nc.vector.tensor_tensor(
                            out=prod[:], in0=_bcast(a_ch[q][:, pxs], 1, 8),
                            in1=ctxd_s[:, j4, c2 * 8:(c2 + 1) * 8, :], op=ALU.mult)
                        for kk in range(8):  # j = j4 + 4k, k = c2*8+kk
                            k = c2 * 8 + kk
                            b = k % 4
                            nc.tensor.matmul(
                                ps_a[k // 4][b * 32:(b + 1) * 32, :],
                                w4s_sb[q][j4][:], prod[:, kk, :],
                                start=False, stop=(q == 1 and j4 == 3 and b == 3),
                                skip_group_check=True, tile_position=(0, b * 32))
            for jq in range(4):
                nc.scalar.activation(out=ctxa_ch[jq][:, pxs], in_=ps_a[jq][:],
                                     func=AF.Copy)
        if DEBUG:
            for jq in range(4):
                nc.sync.dma_start(out=dbg["ctxa_dbg"][jq * 128:(jq + 1) * 128, :],
                                  in_=ctxa_ch[jq][:])

        # ---------------- phase E: Wov -> out1 ----------------
        out1 = [persist.tile([128, NPIX], F32, tag=f"out1_{m}", name=f"out1_{m}") for m in range(NT)]
        for m in range(NT):
            for n in range(2):
                ps = psum.tile([128, 512], F32, tag="mm")
                for k in range(4):
                    nc.tensor.matmul(ps[:], wovt_sb[k][m][:],
                                     ctxa_ch[k][:, n * 512:(n + 1) * 512],
                                     start=(k == 0), stop=(k == 3))
                nc.scalar.activation(out=out1[m][:, n * 512:(n + 1) * 512], in_=ps[:], func=AF.Copy)
        if DEBUG:
            for m in range(NT):
                hi = min(128, CH - m * 128)
                nc.sync.dma_start(out=dbg["out1_dbg"][m * 128:m * 128 + hi, :], in_=out1[m][:hi, :])

        # ---------------- proj_out: gn1/relu/conv1, gn2/relu/conv2 ----------------
        st1 = gn_affine_320(out1, g1_g_sb, g1_b_sb, "gn1")
        pad1 = [persist.tile([128, 34, 34], BF16, tag=f"pad1_{m}", name=f"pad1_{m}") for m in range(NT)]
        for m in range(NT):
            nc.vector.memset(pad1[m][:], 0.0)
            s, t = st1[m]
            nc.scalar.activation(out=pad1[m][:, 1:33, 1:33],
                                 in_=out1[m][:].rearrange("p (h w) -> p h w", w=32),
                                 func=AF.Relu, bias=t[:, 0:1], scale=s[:, 0:1])

        def conv3x3(w_d, src_pad, name):
            cwt = persist.tile([128, 9, 9, 128], BF16, tag="ctxp_rep", name=f"{name}w")
            cw = [[cwt[:, k * 3 + m, :, :] for m in range(NT)] for k in range(NT)]
            for k in range(NT):
                for m in range(NT):
                    nc.sync.dma_start(out=cw[k][m], in_=w_d[k, m])
            out = [persist.tile([128, NPIX], F32, tag=f"out1_{m}", name=f"cv_{m}") for m in range(NT)]
            for m in range(NT):
                for n in range(2):
                    r0 = n * 16
                    ps = psum.tile([128, 512], F32, tag="mm")
                    first = True
                    for tap in range(9):
                        dy, dx = tap // 3, tap % 3
                        for k in range(NT):
                            nc.tensor.matmul(
                                ps[:], cw[k][m][:, tap, :],
                                src_pad[k][:, r0 + dy:r0 + dy + 16, dx:dx + 32],
                                start=first, stop=(tap == 8 and k == NT - 1))
                            first = False
                    nc.scalar.activation(out=out[m][:, n * 512:(n + 1) * 512], in_=ps[:],
                                         func=AF.Copy)
            return out

        y2 = conv3x3(c1_d, pad1, "c1")
        st2 = gn_affine_320(y2, g2_g_sb, g2_b_sb, "gn2")
        pad2 = [persist.tile([128, 34, 34], BF16, tag=f"pad1_{m}", name=f"pad2_{m}") for m in range(NT)]
        for m in range(NT):
            nc.vector.memset(pad2[m][:], 0.0)
            s, t = st2[m]
            nc.scalar.activation(out=pad2[m][:, 1:33, 1:33],
                                 in_=y2[m][:].rearrange("p (h w) -> p h w", w=32),
                                 func=AF.Relu, bias=t[:, 0:1], scale=s[:, 0:1])

        # conv2 + residual (x streamed back in)
        cwt2 = persist.tile([128, 9, 9, 128], BF16, tag="ctxp_rep", name="c2w")
        cw2 = [[cwt2[:, k * 3 + m, :, :] for m in range(NT)] for k in range(NT)]
        for k in range(NT):
            for m in range(NT):
                nc.sync.dma_start(out=cw2[k][m], in_=c2_d[k, m])
        for m in range(NT):
            hi = min(128, CH - m * 128)
            for n in range(2):
                r0 = n * 16
                ps = psum.tile([128, 512], F32, tag="mm")
                first = True
                for tap in range(9):
                    dy, dx = tap // 3, tap % 3
                    for k in range(NT):
                        nc.tensor.matmul(
                            ps[:], cw2[k][m][:, tap, :],
                            pad2[k][:, r0 + dy:r0 + dy + 16, dx:dx + 32],
                            start=first, stop=(tap == 8 and k == NT - 1))
                        first = False
                xres = stage.tile([128, 512], F32, tag="xres", bufs=2)
                nc.sync.dma_start(out=xres[:], in_=x_d[m * 128:(m + 1) * 128,
                                                       n * 512:(n + 1) * 512])
                fin = stage.tile([128, 512], F32, tag="fin", bufs=2)
                nc.vector.tensor_add(fin[:], ps[:], xres[:])
                nc.sync.dma_start(out=y_d[m * 128:m * 128 + hi, n * 512:(n + 1) * 512],
                                  in_=fin[:hi, :])
        es.close()

    nc.compile()
    return nc


_PROG = None
_LAST_RESULTS = None
_LAST_EXEC_NS = None


def _get_prog():
    global _PROG
    if _PROG is None:
        _PROG = build_program()
    return _PROG


def _prep_host(inputs):
    """Precompute folded weights; returns the common (weight) part of in_map."""
    f32 = np.float32
    bf16 = ml_dtypes.bfloat16
    w_in = np.asarray(inputs["w_in"], f32)
    wq = np.asarray(inputs["wq"], f32)
    wk = np.asarray(inputs["wk"], f32)
    wv = np.asarray(inputs["wv"], f32)
    wout = np.asarray(inputs["w_attn_out"], f32)
    pos = np.asarray(inputs["pos_emb"], f32)   # [32 d, 64 c]
    scale = HD ** -0.5

    def pad_to(a, shape):
        out = np.zeros(shape, a.dtype)
        out[tuple(slice(0, s) for s in a.shape)] = a
        return out

    def tile_km(mat_t, kt, mt):  # mat_t: [K, M] -> [kt, mt, 128, 128]
        p = pad_to(mat_t, (kt * 128, mt * 128))
        return np.ascontiguousarray(
            p.reshape(kt, 128, mt, 128).transpose(0, 2, 1, 3))

    w_in_tiles = tile_km(w_in.T, NT, NT).astype(bf16)

    wqk = np.concatenate(
        [scale * (wk[n * HD:(n + 1) * HD, :].T @ wq[n * HD:(n + 1) * HD, :])
         for n in range(HN)], axis=0)          # [512, 320]
    wqkt = pad_to(wqk.T, (NT * 128, 512)).reshape(NT, 128, 512).astype(bf16)

    # sim PSUM row (in tile dh) = 32*((d%16)//4) + 8*(d%4) + 2t + s
    pos2 = np.zeros((2, 4, 128, 128), f32)
    for dh in range(2):
        for t in range(4):
            for s in range(2):
                for d16 in range(16):
                    col = 32 * (d16 // 4) + 8 * (d16 % 4) + 2 * t + s
                    pos2[dh, t, s * 64:(s + 1) * 64, col] = pos[dh * 16 + d16, :]
    # ctxa PSUM row (in tile jq) = 32*((j%16)//4) + 8*(j%4) + 4q + m
    pos4 = np.zeros((2, 4, 128, 128), f32)
    for q in range(2):
        for jq in range(4):
            for m in range(4):
                for j16 in range(16):
                    col = 32 * (j16 // 4) + 8 * (j16 % 4) + 4 * q + m
                    pos4[q, jq, m * 32:(m + 1) * 32, col] = pos[:, jq * 16 + j16]
    # reduce stationaries: block-internal row placement baked into columns
    w2s = np.zeros((4, 4, 128, 32), f32)
    for t in range(4):
        for d4 in range(4):
            for s in range(2):
                w2s[t, d4, s * 64:(s + 1) * 64, d4 * 8 + 2 * t + s] = 1.0
    w4s = np.zeros((2, 4, 128, 32), f32)
    for q in range(2):
        for j4 in range(4):
            for r in range(4):
                w4s[q, j4, r * 32:(r + 1) * 32, j4 * 8 + 4 * q + r] = 1.0

    wov = np.concatenate(
        [wout[:, n * HD:(n + 1) * HD] @ wv[n * HD:(n + 1) * HD, :]
         for n in range(HN)], axis=1)          # [320, 512] cols (n, j)
    # permute cols to ctxa PSUM row layout:
    # k = jq*128 + jmid*32 + j4*8 + 4q + m <-> (n=4q+m, j=jq*16+jmid*4+j4)
    idx = np.zeros(512, np.int64)
    for jq in range(4):
        for jmid in range(4):
            for j4 in range(4):
                for q in range(2):
                    for m in range(4):
                        idx[jq * 128 + jmid * 32 + j4 * 8 + 4 * q + m] = \
                            (4 * q + m) * 64 + jq * 16 + jmid * 4 + j4
    wov_re = wov[:, idx]
    wov_tiles = tile_km(wov_re.T, 4, NT).astype(bf16)

    def conv_tiles(w):  # [o, i, 3, 3] -> [kt, mt, 128, 9, 128]
        taps = np.stack([tile_km(np.ascontiguousarray(w[:, :, t // 3, t % 3].T), NT, NT)
                         for t in range(9)], axis=0)
        return np.ascontiguousarray(taps.transpose(1, 2, 3, 0, 4)).astype(bf16)

    gsel = np.zeros((CH, CH), f32)
    for g in range(8):
        gsel[g * 40:(g + 1) * 40, g * 40:(g + 1) * 40] = 1.0 / 40
    g2ctx = np.zeros((CD, CD), f32)
    for g in range(8):
        g2ctx[g * 8:(g + 1) * 8, g * 8:(g + 1) * 8] = 1.0 / 8

    def col(v):
        return pad_to(np.asarray(v, f32).reshape(-1, 1), (384, 1))

    common = {
        "w_in_t": w_in_tiles,
        "b_in": col(inputs["b_in"]),
        "gin_g": col(inputs["gn_in_g"]), "gin_b": col(inputs["gn_in_b"]),
        "wctx_t": np.ascontiguousarray(np.asarray(inputs["w_ctx"], f32).T).astype(bf16),
        "gctx_g": np.asarray(inputs["gn_ctx_g"], f32).reshape(CD, 1),
        "gctx_b": np.asarray(inputs["gn_ctx_b"], f32).reshape(CD, 1),
        "wqkt": wqkt,
        "pos2": pos2.astype(bf16),
        "pos4": pos4.astype(bf16),
        "w2s": w2s.astype(bf16),
        "w4s": w4s.astype(bf16),
        "wovt": wov_tiles,
        "g1_g": col(inputs["gn1_g"]), "g1_b": col(inputs["gn1_b"]),
        "g2_g": col(inputs["gn2_g"]), "g2_b": col(inputs["gn2_b"]),
        "conv1_t": conv_tiles(np.asarray(inputs["conv1_w"], f32)),
        "conv2_t": conv_tiles(np.asarray(inputs["conv2_w"], f32)),
        "gsel": tile_km(gsel, NT, NT),
        "g2ctx": g2ctx,
    }
    return common


def kernel(**inputs):
    nc = _get_prog()
    common = _prep_host(inputs)
    x = np.asarray(inputs["x"], np.float32)      # [6, 320, 32, 32]
    ctx = np.asarray(inputs["context"], np.float32)  # [6, 64, 32, 32, 32]
    b = x.shape[0]
    in_maps = []
    for core in range(8):
        s = core if core < b else core - b
        m = dict(common)
        xs = np.zeros((384, NPIX), np.float32)
        xs[:CH] = x[s].reshape(CH, NPIX)
        m["x"] = xs
        m["x_bf"] = xs.astype(ml_dtypes.bfloat16)
        m["ctxin"] = np.ascontiguousarray(
            ctx[s].reshape(CD, D * NPIX)).astype(ml_dtypes.bfloat16)
        in_maps.append(m)
    trace = bool(int(os.environ.get("DT_TRACE", "0")))
    kw = {}
    if trace:
        import sys
        import types
        try:
            import antenv.axon_hooks  # noqa: F401
        except ImportError:
            from trn_agent_boot.trn_boot import _ntff_profile_via_ctypes
            mm = types.ModuleType("antenv.axon_hooks")
            _h = _ntff_profile_via_ctypes("/opt/axon/libaxon_pjrt.so")
            mm.get_axon_ntff_profile_hook = lambda: _h
            sys.modules["antenv.axon_hooks"] = mm
        kw = dict(trace=True, tmpdir=os.environ.get("DT_TRACE_DIR") or None)
    res = run_bass_kernel_spmd(nc, in_maps, list(range(8)), **kw)
    global _LAST_RESULTS, _LAST_EXEC_NS
    _LAST_RESULTS = res.results
    _LAST_EXEC_NS = res.exec_time_ns
    if trace:
        print(f"HW exec time: {res.exec_time_ns} ns")
    out = np.stack([res.results[s]["y"] for s in range(b)], axis=0)
    return out.reshape(b, CH, 32, 32).astype(np.float32)


if __name__ == "__main__":
    pass
